# revision 9
# baseline (speedup 1.0000x reference)
"""Trainium2 Bass kernel for masked/weighted multi-head attention.

Problem (nn_MultiHeadAttention): B=4, N=M=2048, C=512, H=8, DH=64.
  q = input_q @ Wq + bq ; k,v likewise
  scores = (q @ k^T) / sqrt(DH) * attention_factors * key_weights
  scores = -inf where key_masks | attention_masks
  attn   = softmax(scores, axis=-1)          -> (B, H, N, M) fp32
  hidden = attn @ v                          -> (B, N, C) fp32

Sharding: 8 cores = (batch b = core//2) x (query-row half = core%2).
Each core handles all 8 heads for its 1024 query rows. No collectives:
inputs are sliced on host, outputs gathered on host.

Per-core dataflow (orientation: n on partitions, m on free dim):
  - transpose inputs on PE (fp32), project qT/kT (c-major) and v (m-major)
  - scores: S[n-tile, m] = qT8_h^T @ kTw_h  (fp32r, 1/8 folded into q,
    key_weights folded into kT)
  - T = S * attention_factors        (DVE tensor_tensor, PSUM source)
  - E = exp(T)                       (ACT, in-place)
  - P = E * KEEP, rowsum = sum_m P   (DVE scalar_tensor_tensor + accum;
    KEEP = (~am & ~km) as bf16 {0,1}, exact)
  - A = P * (1/rowsum)               (per-partition tensor_scalar, 2x mode)
  - attn out: DMA A (fp32, exact softmax)
  - PV: cast A -> fp16, PE-transpose 128x128 tiles, hiddenT_h = sum_mc
    v16_h[mc]^T @ A16^T[mc]  (fp16 matmul, fp32 accum in PSUM)
  - hidden written transposed per head ([h, dh, n]); host fixes layout.
"""

import os
import sys

import numpy as np

if "/opt/trn_rl_repo" not in sys.path:
    sys.path.insert(0, "/opt/trn_rl_repo")

# Problem constants (hardcoded per harness contract)
B, N, M, C, H = 4, 2048, 2048, 512, 8
DH = C // H
NCORES = 8
N_HALF = N // 2

_CACHE = {}


def _build(n_half=N_HALF, m=M, c=C, h=H):
    """Build the per-core Bass program (same NEFF for all 8 cores)."""
    from contextlib import ExitStack

    import concourse.bacc as bacc
    import concourse.mybir as mybir
    import concourse.tile as tile
    from concourse.bass import ts
    from concourse.masks import make_identity

    dh = c // h
    P = 128
    NT = n_half // P          # n tiles (8)
    MC = m // P               # m chunks of 128 (16)
    MS = m // 512             # m slices of 512 (4)
    CIN = c // P              # input-channel chunks (4)
    CC = c // P               # output-channel chunks (4)
    HPC = P // dh             # heads per 128-chunk (2)

    f32 = mybir.dt.float32
    f32r = mybir.dt.float32r
    f16 = mybir.dt.float16
    bf16 = mybir.dt.bfloat16
    u8 = mybir.dt.uint8
    Alu = mybir.AluOpType
    Act = mybir.ActivationFunctionType

    nc = bacc.Bacc("TRN2", target_bir_lowering=False, debug=False,
                   enable_asserts=False)

    # ---- DRAM I/O ----
    inq = nc.dram_tensor("inq", (n_half, c), f32, kind="ExternalInput").ap()
    ink = nc.dram_tensor("ink", (m, c), f32, kind="ExternalInput").ap()
    inv = nc.dram_tensor("inv", (m, c), f32, kind="ExternalInput").ap()
    wq_d = nc.dram_tensor("wq", (c, c), f32r, kind="ExternalInput").ap()
    wk_d = nc.dram_tensor("wk", (c, c), f32r, kind="ExternalInput").ap()
    wv_d = nc.dram_tensor("wv", (c, c), f32r, kind="ExternalInput").ap()
    bq_d = nc.dram_tensor("bq", (c,), f32, kind="ExternalInput").ap()
    bk_d = nc.dram_tensor("bk", (c,), f32, kind="ExternalInput").ap()
    bv_d = nc.dram_tensor("bv", (c,), f32, kind="ExternalInput").ap()
    kw_d = nc.dram_tensor("kw", (m,), f32, kind="ExternalInput").ap()
    km_d = nc.dram_tensor("km", (m,), u8, kind="ExternalInput").ap()
    af_d = nc.dram_tensor("af", (n_half, m), f32, kind="ExternalInput").ap()
    am_d = nc.dram_tensor("am", (n_half, m), u8, kind="ExternalInput").ap()
    attn_o = nc.dram_tensor("attn_o", (h, n_half, m), f32,
                            kind="ExternalOutput").ap()
    hid_o = nc.dram_tensor("hid_o", (h, dh, n_half), f32,
                           kind="ExternalOutput").ap()

    with tile.TileContext(nc) as tc, ExitStack() as ctx:
        const = ctx.enter_context(tc.tile_pool(name="const", bufs=1))

        ident32 = const.tile([P, P], f32)
        make_identity(nc, ident32)
        ident16 = const.tile([P, P], f16)
        nc.vector.tensor_copy(ident16, ident32)
        ones_row = const.tile([1, P], f32)
        nc.vector.memset(ones_row, 1.0)

        # persistent big tensors
        big = ctx.enter_context(tc.tile_pool(name="big", bufs=1))
        qT8 = [big.tile([P, n_half], f32r, name=f"qT8_{i}") for i in range(CC)]
        kTw = [big.tile([P, m], f32r, name=f"kTw_{i}") for i in range(CC)]
        v16 = [big.tile([P, c], f16, name=f"v16_{i}") for i in range(MC)]
        kmkb = big.tile([P, m], f32)

        setup = ctx.enter_context(tc.tile_pool(name="setup", bufs=1))
        # per-m row vectors (freed when setup pool slots are reused later)
        kw_row = setup.tile([1, m], f32)
        nc.sync.dma_start(kw_row, kw_d[None, :])
        kmk_row = setup.tile([1, m], f32)
        nc.gpsimd.dma_start(kmk_row, km_d[None, :])  # u8 -> f32 cast
        nc.vector.tensor_single_scalar(kmk_row, kmk_row, 0.0, Alu.is_equal)

        # biases: [128, CC] with bias for chunk cc in column cc
        bq8 = setup.tile([P, CC], f32)
        nc.sync.dma_start(bq8, bq_d.rearrange("(cc p) -> p cc", p=P))
        nc.vector.tensor_scalar_mul(bq8, bq8, 0.125)
        bkt = setup.tile([P, CC], f32)
        nc.sync.dma_start(bkt, bk_d.rearrange("(cc p) -> p cc", p=P))
        bv_row = setup.tile([1, c], f32)
        nc.sync.dma_start(bv_row, bv_d[None, :])
        bv_bc = setup.tile([P, c], f32)

        # ---------- stage 0: broadcasts of per-m vectors ----------
        with tc.tile_pool(name="bc_ps", bufs=1, space="PSUM") as bcps:
            kmkb_ps = bcps.tile([P, m], f32)
            for msl in range(MS):
                nc.tensor.matmul(kmkb_ps[:, ts(msl, 512)], lhsT=ones_row,
                                 rhs=kmk_row[:, ts(msl, 512)],
                                 start=True, stop=True)
            nc.scalar.copy(kmkb, kmkb_ps)
            bv_ps = bcps.tile([P, c], f32)
            for csl in range(c // 512):
                nc.tensor.matmul(bv_ps[:, ts(csl, 512)], lhsT=ones_row,
                                 rhs=bv_row[:, ts(csl, 512)],
                                 start=True, stop=True)
            nc.scalar.copy(bv_bc, bv_ps)

        def transpose_input(xdram, rows, xT, xpool, tpps):
            # xT: list of CIN tiles [128, rows]
            for g in range(rows // 512):
                tps = [tpps.tile([P, 512], f32, name=f"tp_{cin}",
                                 tag=f"tp_{cin}") for cin in range(CIN)]
                for j in range(4):
                    r = g * 4 + j
                    x_sb = xpool.tile([P, c], f32, name="x_sb", tag="x_sb")
                    nc.sync.dma_start(x_sb, xdram[ts(r, P), :])
                    for cin in range(CIN):
                        nc.tensor.transpose(tps[cin][:, ts(j, P)],
                                            x_sb[:, ts(cin, P)], ident32)
                for cin in range(CIN):
                    if (g + cin) % 2 == 0:
                        nc.scalar.copy(xT[cin][:, ts(g, 512)], tps[cin])
                    else:
                        nc.vector.tensor_copy(xT[cin][:, ts(g, 512)], tps[cin])

        # ---------- stage A/B: transpose inputs + projections ----------
        with tc.tile_pool(name="xT", bufs=1) as xTp, \
             tc.tile_pool(name="w_sb", bufs=1) as wp, \
             tc.tile_pool(name="xload", bufs=4) as xload, \
             tc.tile_pool(name="tp_ps", bufs=1, space="PSUM") as tpps, \
             tc.tile_pool(name="proj_ps", bufs=2, space="PSUM") as pps:

            w_sb = [wp.tile([P, c], f32r, name=f"w_sb_{i}") for i in range(CIN)]

            def load_w(wd):
                for cin in range(CIN):
                    nc.sync.dma_start(w_sb[cin], wd[ts(cin, P), :])

            # ---- Q ----
            inqT = [xTp.tile([P, n_half], f32r, name=f"inqT_{i}",
                             tag=f"xT_{i}") for i in range(CIN)]
            transpose_input(inq, n_half, inqT, xload, tpps)
            load_w(wq_d)
            for cc in range(CC):
                for nb in range(n_half // 512):
                    ps = pps.tile([P, 512], f32, name="proj", tag="proj")
                    for cin in range(CIN):
                        nc.tensor.matmul(
                            ps, lhsT=w_sb[cin][:, ts(cc, P)],
                            rhs=inqT[cin][:, ts(nb, 512)],
                            start=(cin == 0), stop=(cin == CIN - 1))
                    nc.scalar.activation(qT8[cc][:, ts(nb, 512)], ps,
                                         Act.Identity, bias=bq8[:, cc:cc + 1],
                                         scale=0.125)

            # ---- K ----
            inkT = [xTp.tile([P, m], f32r, name=f"inkT_{i}",
                             tag=f"xT_{i}") for i in range(CIN)]
            transpose_input(ink, m, inkT, xload, tpps)
            load_w(wk_d)
            for cc in range(CC):
                for nb in range(m // 512):
                    ps = pps.tile([P, 512], f32, name="proj", tag="proj")
                    for cin in range(CIN):
                        nc.tensor.matmul(
                            ps, lhsT=w_sb[cin][:, ts(cc, P)],
                            rhs=inkT[cin][:, ts(nb, 512)],
                            start=(cin == 0), stop=(cin == CIN - 1))
                    nc.scalar.activation(kTw[cc][:, ts(nb, 512)], ps,
                                         Act.Identity, bias=bkt[:, cc:cc + 1],
                                         scale=1.0)
            # ---- V ----
            invT = [xTp.tile([P, m], f32r, name=f"invT_{i}",
                             tag=f"xT_{i}") for i in range(CIN)]
            transpose_input(inv, m, invT, xload, tpps)
            load_w(wv_d)
            for mc in range(MC):
                ps = pps.tile([P, c], f32, name="projv", tag="proj")
                for cin in range(CIN):
                    nc.tensor.matmul(
                        ps, lhsT=invT[cin][:, ts(mc, P)],
                        rhs=w_sb[cin],
                        start=(cin == 0), stop=(cin == CIN - 1))
                nc.vector.affine_then_add(v16[mc], ps, bv_bc, 1.0, 0.0)

            # fold key_weights into kT (broadcast kw over partitions via PE)
            with tc.tile_pool(name="kw_ps", bufs=2, space="PSUM") as kwps:
                for msl in range(MS):
                    kwb_ps = kwps.tile([P, 512], f32, name="kwb", tag="kwb")
                    nc.tensor.matmul(kwb_ps, lhsT=ones_row,
                                     rhs=kw_row[:, ts(msl, 512)],
                                     start=True, stop=True)
                    for cc in range(CC):
                        nc.vector.tensor_tensor(kTw[cc][:, ts(msl, 512)],
                                                kTw[cc][:, ts(msl, 512)],
                                                kwb_ps, Alu.mult)

        # ---------- stage D: hot loop (KEEP built per n-tile) ----------
        with tc.tile_pool(name="af", bufs=2) as afp, \
             tc.tile_pool(name="keep", bufs=2) as keepp, \
             tc.tile_pool(name="chain", bufs=3) as chp, \
             tc.tile_pool(name="a16", bufs=2) as a16p, \
             tc.tile_pool(name="atsb", bufs=4) as atsbp, \
             tc.tile_pool(name="hts", bufs=2) as htsp, \
             tc.tile_pool(name="rs", bufs=8) as rsp, \
             tc.tile_pool(name="s_ps", bufs=1, space="PSUM") as sps, \
             tc.tile_pool(name="at_ps", bufs=1, space="PSUM") as atps, \
             tc.tile_pool(name="ht_ps", bufs=2, space="PSUM") as htps:

            for nt in range(NT):
                af_t = afp.tile([P, m], f32, name="af_t", tag="af_t")
                nc.sync.dma_start(af_t, af_d[ts(nt, P), :])
                amf = keepp.tile([P, m], bf16, name="amf", tag="amf")
                nc.gpsimd.dma_start(amf, am_d[ts(nt, P), :])  # u8->bf16
                keep_t = keepp.tile([P, m], bf16, name="keep_t", tag="keep_t")
                nc.vector.scalar_tensor_tensor(keep_t, amf, 0.0, kmkb,
                                               Alu.is_equal, Alu.mult)
                for hh in range(h):
                    cc = hh // HPC
                    ro = (hh % HPC) * dh
                    # scores -> PSUM [128, m]
                    s_ps = sps.tile([P, m], f32, name="s", tag="s")
                    for msl in range(MS):
                        nc.tensor.matmul(
                            s_ps[:, ts(msl, 512)],
                            lhsT=qT8[cc][ro:ro + dh, ts(nt, P)],
                            rhs=kTw[cc][ro:ro + dh, ts(msl, 512)],
                            start=True, stop=True)
                    chain = chp.tile([P, m], f32, name="chain", tag="chain")
                    nc.vector.tensor_tensor(chain, s_ps, af_t, Alu.mult)
                    nc.scalar.activation(chain, chain, Act.Exp)
                    rs = rsp.tile([P, 1], f32, name="rs", tag="rs")
                    nc.vector.scalar_tensor_tensor(chain, chain, 1.0,
                                                   keep_t, Alu.mult,
                                                   Alu.mult, accum_out=rs)
                    rcp = rsp.tile([P, 1], f32, name="rcp", tag="rcp")
                    nc.vector.reciprocal(rcp, rs)
                    nc.vector.tensor_scalar_mul(chain, chain, rcp)
                    nc.sync.dma_start(attn_o[hh, ts(nt, P), :], chain)
                    # PV path: cast to fp16, transpose, matmul
                    a16 = a16p.tile([P, m], f16, name="a16", tag="a16")
                    nc.gpsimd.tensor_copy(a16, chain)
                    ht_ps = htps.tile([dh, P], f32, name="ht", tag="ht")
                    for mg in range(MC // 4):
                        at_ps = atps.tile([P, 512], f16, name="at", tag="at")
                        for j in range(4):
                            mc = mg * 4 + j
                            nc.tensor.transpose(at_ps[:, ts(j, P)],
                                                a16[:, ts(mc, P)], ident16)
                        at_sb = atsbp.tile([P, 512], f16, name="at_sb",
                                           tag="at_sb")
                        if mg % 2 == 0:
                            nc.vector.tensor_copy(at_sb, at_ps)
                        else:
                            nc.scalar.copy(at_sb, at_ps)
                        for j in range(4):
                            mc = mg * 4 + j
                            nc.tensor.matmul(
                                ht_ps,
                                lhsT=v16[mc][:, hh * dh:(hh + 1) * dh],
                                rhs=at_sb[:, ts(j, P)],
                                start=(mc == 0), stop=(mc == MC - 1))
                    hts = htsp.tile([dh, P], f32, name="hts", tag="hts")
                    nc.scalar.copy(hts, ht_ps)
                    nc.sync.dma_start(hid_o[hh, :, ts(nt, P)], hts)

    nc.compile()
    return nc


def _get_program():
    key = "full"
    if key not in _CACHE:
        _CACHE[key] = _build()
    return _CACHE[key]


def _make_in_maps(inputs):
    """Slice full inputs into 8 per-core input dicts."""
    iq = np.asarray(inputs["input_q"], dtype=np.float32)
    ik = np.asarray(inputs["input_k"], dtype=np.float32)
    iv = np.asarray(inputs["input_v"], dtype=np.float32)
    kw = np.asarray(inputs["key_weights"], dtype=np.float32)
    km = np.asarray(inputs["key_masks"]).astype(np.uint8)
    af = np.asarray(inputs["attention_factors"], dtype=np.float32)
    am = np.asarray(inputs["attention_masks"]).astype(np.uint8)
    wq = np.asarray(inputs["Wq"], dtype=np.float32)
    wk = np.asarray(inputs["Wk"], dtype=np.float32)
    wv = np.asarray(inputs["Wv"], dtype=np.float32)
    bq = np.asarray(inputs["bq"], dtype=np.float32)
    bk = np.asarray(inputs["bk"], dtype=np.float32)
    bv = np.asarray(inputs["bv"], dtype=np.float32)

    in_maps = []
    for core in range(NCORES):
        b, nh = core // 2, core % 2
        n0 = nh * N_HALF
        in_maps.append({
            "inq": np.ascontiguousarray(iq[b, n0:n0 + N_HALF]),
            "ink": np.ascontiguousarray(ik[b]),
            "inv": np.ascontiguousarray(iv[b]),
            "wq": wq, "wk": wk, "wv": wv,
            "bq": bq, "bk": bk, "bv": bv,
            "kw": np.ascontiguousarray(kw[b]),
            "km": np.ascontiguousarray(km[b]),
            "af": np.ascontiguousarray(af[b, n0:n0 + N_HALF]),
            "am": np.ascontiguousarray(am[b, n0:n0 + N_HALF]),
        })
    return in_maps


def _gather(results):
    attn = np.empty((B, H, N, M), dtype=np.float32)
    hidden = np.empty((B, N, C), dtype=np.float32)
    for core in range(NCORES):
        b, nh = core // 2, core % 2
        n0 = nh * N_HALF
        res = results[core]
        attn[b, :, n0:n0 + N_HALF, :] = res["attn_o"]
        # hid_o: [H, DH, N_HALF] -> [N_HALF, H*DH]
        hidden[b, n0:n0 + N_HALF, :] = (
            res["hid_o"].transpose(2, 0, 1).reshape(N_HALF, C))
    return hidden, attn


def run_spmd(inputs, trace=False, **kwargs):
    from concourse.bass_utils import run_bass_kernel_spmd

    nc = _get_program()
    in_maps = _make_in_maps(inputs)
    out = run_bass_kernel_spmd(nc, in_maps, core_ids=list(range(NCORES)),
                               trace=trace, **kwargs)
    return out


def kernel(**inputs):
    out = run_spmd(inputs, trace=False)
    return _gather(out.results)
